# revision 1
# baseline (speedup 1.0000x reference)
"""Trainium2 Bass kernel for nn_LCN (locally-connected network).

Computation:
  x: (512, 1, 280, 280) -> non-overlapping 28x28 patches (10x10 grid, P=100)
  y[b, f, p] = sum_{k,l} x[b, 28ph+k, 28pw+l] * w[f*100+p, 0, k, l]
  y = relu(y + bias[f*100+p]);  out = y_flat @ dec_w.T + dec_b   (j = f*100 + p)

Sharding: 8 cores = 4 batch groups x 2 image halves (rows 0..139 | 140..279).
Each core: 128 images, 5 bands (28 rows each), 50 patches. The problem is
DMA-stream-bound (~20 MB of fp32 x per core at ~360 GB/s), so everything
is organized as a software pipeline against the just-in-time HBM stream:

  - x streams in 2 chunk DMAs per band (14 k-rows each; 15,680 B/partition
    keeps per-DMA queue cost low), prefetched 3 bands deep.
  - DVE/ACT im2col: reorder band [b,(k pw l)] -> patch-major [b,(pw k l)],
    casting fp32 -> bf16 (bf16 runs the PE at 1 cyc/row vs 4 for fp32; the
    rel-err budget 2e-2 >> bf16's ~5e-3). Done in pw-pair pieces in slots
    scheduled so each PE step gates on a ~0.7us copy, not a chunk.
  - PE transposes per t-group (t0-2 | t3-4 | t5-6, aligned to the chunk
    split) in patch pair/quad groups into one PSUM bank tile; single
    DVE/ACT copy evacuates it to SBUF (amortizes PSUM-access init).
  - Mains: per patch 7 accumulating bf16 matmuls lhsT=w[112,16],
    rhs=xT[112,128] -> y PSUM (4 patches per tile at offsets 0/32/64/96),
    emitted 2 steps behind the transposes (depth-2 pipeline) so evacs
    never stall PE (keeps its p-state clock at max).
  - ACT: relu(y + bias) -> y_sb bf16; decoder matmul for each group is
    emitted inline right after its relu (nothing decoder-sized remains in
    the tail).
Host sums the two half-image partial decoder outputs and adds dec_b.
"""

import sys

import numpy as np

for _p in ("/opt/trn_rl_repo", "/opt/trn_rl_repo/concourse"):
    if _p not in sys.path:
        sys.path.insert(0, _p)

import concourse.bass as bass
import concourse.mybir as mybir
import concourse.tile as tile
from concourse import bacc
from concourse.masks import make_identity

F32 = mybir.dt.float32

# Problem constants
B, H, W = 512, 280, 280
KS = 28
HS = WS = 10
F = 16
OUT = 10
NCORES = 8
BLOC = 128      # images per core
NBANDS = 5      # bands per core (half image)
NPW = 10        # patches per band
NCHUNK = 7      # 112-pixel chunks per patch (4 rows x 28 cols each)
CK = 112        # contraction chunk size
BAND_W = KS * W  # 7840 elements per band per image


# DMA chunks per band, k-rows each. 14/14 keeps the per-DMA fixed cost
# low (10 x-DMAs; each 15,680 B/partition) — finer chunking measurably
# inflates per-queue DMA busy more than the smoother arrivals save.
CHUNK_K = [14, 14]              # k0-13 | k14-27
CHUNK_KOFF = [0, 14]
# transpose-group phases per band: (t_lo, t_hi, patch groups rel. to band)
TGROUPS = [
    (0, 3, [(0, 1), (2, 3), (4, 5), (6, 7), (8, 9)]),
    (3, 5, [(0, 1, 2, 3), (4, 5, 6, 7), (8, 9)]),
    (5, 7, [(0, 1, 2, 3), (4, 5, 6, 7), (8, 9)]),
]


def build_program(n_bands=NBANDS, n_pw=NPW, use_is_transpose=True, use_bf16=False):
    np_loc = n_bands * n_pw
    ng = (np_loc + 3) // 4
    WDT = mybir.dt.bfloat16 if use_bf16 else F32
    YB = 3                  # y PSUM bufs (max concurrently-open groups)

    nc = bacc.Bacc("TRN2")
    x_d = nc.dram_tensor("x", [BLOC, n_bands * BAND_W], F32, kind="ExternalInput")
    w_d = nc.dram_tensor("w", [CK, np_loc * NCHUNK * F], WDT, kind="ExternalInput")
    b_d = nc.dram_tensor("bias", [128, ng], F32, kind="ExternalInput")
    d_d = nc.dram_tensor("dec", [128, ng * OUT], WDT, kind="ExternalInput")
    o_d = nc.dram_tensor("out", [OUT, BLOC], F32, kind="ExternalOutput")

    with tile.TileContext(nc) as tc:
        with (
            tc.tile_pool(name="const", bufs=1) as constp,
            tc.tile_pool(name="xc0", bufs=3) as xc0p,
            tc.tile_pool(name="xc12", bufs=3) as xc12p,
            tc.tile_pool(name="xpm", bufs=2) as xpmp,
            tc.tile_pool(name="xt", bufs=4) as xtp,
            tc.tile_pool(name="xtps", bufs=4, space="PSUM") as xtpsp,
            tc.tile_pool(name="yps", bufs=YB, space="PSUM") as ypsp,
            tc.tile_pool(name="ops", bufs=1, space="PSUM") as opsp,
        ):
            ident = constp.tile([128, 128], WDT)
            make_identity(nc, ident[:])
            zero_sb = constp.tile([128, 128], F32)
            nc.gpsimd.memset(zero_sb[:], 0.0)
            w_sb = constp.tile([CK, np_loc * NCHUNK * F], WDT)
            bias_sb = constp.tile([128, ng], F32)
            dec_sb = constp.tile([128, ng * OUT], WDT)
            y_sb = constp.tile([128, ng * 128], WDT)
            out_ps = opsp.tile([OUT, BLOC], F32)

            chunk_tiles = {}
            xpm_tiles = {}

            def load_chunk(b, s):
                kc = CHUNK_K[s]
                pool = xc0p if s == 0 else xc12p
                t = pool.tile([128, kc * W], F32, name=f"x_c{min(s, 1)}")
                off = b * BAND_W + CHUNK_KOFF[s] * W
                nc.sync.dma_start(out=t[:], in_=x_d[:, off:off + kc * W])
                chunk_tiles[(b, s)] = t

            def im2col(b, s, pw0, pw1, eng):
                # reorder chunk columns for patches [pw0, pw1) -> patch-
                # major region of x_pm [b, (pw k l)], casting fp32 -> bf16.
                # pw-wise pieces let each PE step gate on one small copy
                # instead of a whole chunk's reorder.
                if b not in xpm_tiles:
                    xpm_tiles[b] = xpmp.tile([128, BAND_W], WDT, name="x_pm")
                t = xpm_tiles[b]
                dst4 = t[:].rearrange("b (pw k l) -> b pw k l", pw=n_pw, k=KS)
                src4 = chunk_tiles[(b, s)][:].rearrange(
                    "b (k pw l) -> b pw k l", k=CHUNK_K[s], pw=n_pw)
                kg = CHUNK_KOFF[s]
                dst = dst4[:, pw0:pw1, kg:kg + CHUNK_K[s], :]
                src = src4[:, pw0:pw1, :, :]
                if eng == "act":
                    nc.scalar.activation(
                        out=dst, in_=src,
                        func=mybir.ActivationFunctionType.Copy)
                else:
                    nc.vector.tensor_copy(dst, src)


            y_tiles = {}

            def emit_mms(group, t_lo, t_hi, xt):
                nt = t_hi - t_lo
                for idx, pp in enumerate(group):
                    G, q = pp // 4, pp % 4
                    if G not in y_tiles:
                        yt = ypsp.tile([128, 128], F32, name="y_ps")
                        if G < YB:
                            # clear stale/NaN PSUM so gaps are finite
                            nc.vector.tensor_copy(yt[:], zero_sb[:])
                        y_tiles[G] = yt
                    yt = y_tiles[G]
                    for t in range(t_lo, t_hi):
                        rhs = xt[:, (idx * nt + t - t_lo) * 128:
                                 (idx * nt + t - t_lo + 1) * 128]
                        nc.tensor.matmul(
                            yt[32 * q:32 * q + F, :],
                            w_sb[:, (pp * NCHUNK + t) * F:
                                 (pp * NCHUNK + t + 1) * F],
                            rhs,
                            start=(t == 0),
                            stop=(t == NCHUNK - 1),
                            tile_position=(0, 32 * q),
                        )
                    if t_hi == NCHUNK and (q == 3 or pp == np_loc - 1):
                        nc.scalar.activation(
                            out=y_sb[:, G * 128:(G + 1) * 128],
                            in_=yt[:],
                            func=mybir.ActivationFunctionType.Relu,
                            bias=bias_sb[:, G:G + 1],
                        )
                        y_tiles.pop(G)
                        # decoder accumulation inline: keeps it off the tail
                        nc.tensor.matmul(
                            out_ps[:],
                            dec_sb[:, G * OUT:(G + 1) * OUT],
                            y_sb[:, G * 128:(G + 1) * 128],
                            start=(G == 0),
                            stop=(G == ng - 1),
                        )

            def transposes(group, t_lo, t_hi, evac_eng):
                # transpose the group's patches for t-range into one PSUM
                # tile, then evacuate with a single DVE copy (amortizes
                # PSUM-access init). Uniform full-bank tiles [CK, 1024].
                band = group[0] // n_pw
                x_pm = xpm_tiles[band]
                nt = t_hi - t_lo
                ps = xtpsp.tile([CK, 1024], WDT, name="xt_ps")
                sb = xtp.tile([CK, 1024], WDT, name="xt")
                for idx, p in enumerate(group):
                    pw = p % n_pw
                    for t in range(t_lo, t_hi):
                        col = idx * nt + t - t_lo
                        dst = ps[:, col * 128:(col + 1) * 128]
                        src = x_pm[:, pw * 784 + t * CK: pw * 784 + (t + 1) * CK]
                        nc.tensor.transpose(dst, src, ident[:])
                ncols = len(group) * nt * 128
                if evac_eng == "act":
                    nc.scalar.activation(
                        out=sb[:, :ncols], in_=ps[:, :ncols],
                        func=mybir.ActivationFunctionType.Copy)
                else:
                    nc.vector.tensor_copy(sb[:, :ncols], ps[:, :ncols])
                return sb

            # depth-2 software pipeline: mains for group i are emitted after
            # the transposes of group i+2, giving the DVE evac two full
            # transpose groups of PE time to complete in (depth-3 was tried
            # and fails: it overruns the 4-buf xt/PSUM rotation)
            pending = []

            def step(group, t_lo, t_hi, evac_eng="vec"):
                xt = transposes(group, t_lo, t_hi, evac_eng)
                pending.append((group, t_lo, t_hi, xt))
                if len(pending) > 2:
                    emit_mms(*pending.pop(0))

            WB = n_pw * NCHUNK * F  # w_sb columns per band

            # weights/bias/dec go on the second HWDGE ring (ACT) so they
            # never sit in front of x chunks in the sync-ring FIFO
            def load_w(b):
                nc.sync.dma_start(
                    out=w_sb[:, b * WB:(b + 1) * WB],
                    in_=w_d[:, b * WB:(b + 1) * WB])

            def load_w_rest():
                nc.sync.dma_start(
                    out=w_sb[:, WB:], in_=w_d[:, WB:])

            for band in range(n_bands):
                p0 = band * n_pw
                if band == 0:
                    # w streams in per-band slices tucked after each c0 so
                    # no w transfer ever sits in front of a chunk the PE
                    # pipeline is about to gate on (a single consolidated
                    # w DMA created a 2.8us stream lump -> 4us PE stall)
                    load_chunk(0, 0)
                    load_w(0)
                    load_chunk(0, 1)
                    nc.sync.dma_start(out=bias_sb[:], in_=b_d[:])
                    if n_bands > 1:
                        load_chunk(1, 0)
                        load_w(1)
                        load_chunk(1, 1)
                    if n_bands > 2:
                        load_chunk(2, 0)
                        load_w(2)
                        load_chunk(2, 1)
                    nc.sync.dma_start(out=dec_sb[:], in_=d_d[:])
                    # band 0 startup: c0 pieces feed the first A steps
                    im2col(0, 0, 0, 2, "act")
                    im2col(0, 0, 2, 4, "vec")
                    im2col(0, 0, 4, 6, "act")
                # im2col slots: chunks arrive just-in-time; pw-pair pieces
                # gate each PE step on a ~0.7us copy instead of a whole
                # chunk reorder, in slots that don't head-block evacs PE
                # is about to need.
                for gi, (t_lo, t_hi, groups) in enumerate(TGROUPS):
                    for si, g in enumerate(groups):
                        if gi == 0:
                            if si == 0:
                                im2col(band, 0, 6, 8, "vec")
                            elif si == 1:
                                im2col(band, 0, 8, 10, "act")
                            elif si == 2:
                                if band + 3 < n_bands:
                                    load_chunk(band + 3, 0)
                                    load_w(band + 3)
                                    load_chunk(band + 3, 1)
                                im2col(band, 1, 0, 2, "act")
                            elif si == 3:
                                im2col(band, 1, 2, 4, "vec")
                            elif si == 4:
                                im2col(band, 1, 4, 6, "act")
                        elif gi == 1:
                            if si == 0:
                                im2col(band, 1, 6, 8, "act")
                            elif si == 1:
                                im2col(band, 1, 8, 10, "act")
                        else:
                            if si == 0 and band + 1 < n_bands:
                                im2col(band + 1, 0, 0, 2, "act")
                            elif si == 1 and band + 1 < n_bands:
                                im2col(band + 1, 0, 2, 4, "vec")
                            elif si == 2 and band + 1 < n_bands:
                                im2col(band + 1, 0, 4, 6, "act")
                        # second quad of each B t-group evacs on ACT to
                        # relieve DVE (its per-band load is near the
                        # stream cadence)
                        eng = "act" if gi > 0 and si == 1 else "vec"
                        step(tuple(p0 + i for i in g), t_lo, t_hi, eng)
            while pending:
                emit_mms(*pending.pop(0))

            out_sb = constp.tile([OUT, BLOC], F32)
            nc.vector.tensor_copy(out_sb[:], out_ps[:])
            nc.sync.dma_start(out=o_d[:], in_=out_sb[:])

    return nc


def stage_half(weight, bias, dec_w, h, n_bands=NBANDS, n_pw=NPW):
    """Host-side staging of weights/bias/decoder for image-half h (0 or 1)."""
    np_loc = n_bands * n_pw
    ng = (np_loc + 3) // 4
    weight = np.asarray(weight, np.float32)
    bias = np.asarray(bias, np.float32)
    dec_w = np.asarray(dec_w, np.float32)

    # w: (1600, 1, 28, 28) -> [f, ph, pw, k, l] -> chunks [d=(kk,l), (bl,pw,t,f)]
    w5 = weight.reshape(F, HS, WS, KS, KS)[:, n_bands * h:n_bands * h + n_bands]
    w6 = w5.reshape(F, n_bands, WS, NCHUNK, 4, KS)  # f bl pw t kk l
    wst = np.ascontiguousarray(
        np.transpose(w6, (4, 5, 1, 2, 3, 0))).reshape(CK, np_loc * NCHUNK * F)

    b5 = bias.reshape(F, HS, WS)[:, n_bands * h:n_bands * h + n_bands, :]
    b5 = b5.reshape(F, np_loc)
    bst = np.zeros((128, ng), np.float32)
    d5 = dec_w.reshape(OUT, F, HS, WS)[:, :, n_bands * h:n_bands * h + n_bands, :]
    d5 = d5.reshape(OUT, F, np_loc)
    dst_ = np.zeros((128, ng * OUT), np.float32)
    for pl in range(np_loc):
        G, q = pl // 4, pl % 4
        bst[32 * q:32 * q + F, G] = b5[:, pl]
        dst_[32 * q:32 * q + F, G * OUT:(G + 1) * OUT] = d5[:, :, pl].T
    return wst, bst, dst_


_cache = {}
USE_BF16 = True
USE_IS_TRANSPOSE = True


def _get_nc():
    key = ("nc", USE_BF16, USE_IS_TRANSPOSE)
    if key not in _cache:
        nc = build_program(use_is_transpose=USE_IS_TRANSPOSE, use_bf16=USE_BF16)
        nc.finalize()
        _cache[key] = nc
    return _cache[key]


def make_in_maps(x, weight, bias, dec_w):
    x = np.asarray(x, np.float32)
    stages = [stage_half(weight, bias, dec_w, h) for h in (0, 1)]
    in_maps = []
    for core in range(NCORES):
        bg, h = core // 2, core % 2
        xs = np.ascontiguousarray(
            x[bg * BLOC:(bg + 1) * BLOC, 0, 140 * h:140 * h + 140, :]
        ).reshape(BLOC, NBANDS * BAND_W)
        wst, bst, dst_ = stages[h]
        if USE_BF16:
            import ml_dtypes
            wst = wst.astype(ml_dtypes.bfloat16)
            dst_ = dst_.astype(ml_dtypes.bfloat16)
        in_maps.append({"x": xs, "w": wst, "bias": bst, "dec": dst_})
    return in_maps


def combine(results, dec_b):
    out = np.zeros((B, OUT), np.float32)
    for bg in range(4):
        part = results[2 * bg]["out"] + results[2 * bg + 1]["out"]  # (10, 128)
        out[bg * BLOC:(bg + 1) * BLOC] = part.T + np.asarray(dec_b, np.float32)
    return out


def _install_ntff_hook():
    """Provide the missing antenv.axon_hooks module so trace=True works
    under axon (replicates trn_boot._ntff_profile_via_ctypes)."""
    import contextlib
    import ctypes
    import types

    if "antenv.axon_hooks" in sys.modules:
        return
    so_path = "/opt/axon/libaxon_pjrt.so"
    holder = {}
    mod = types.ModuleType("antenv.axon_hooks")
    mod.set_axon_ntff_profile_hook = lambda h: holder.__setitem__("h", h)
    mod.get_axon_ntff_profile_hook = lambda: holder.get("h")
    sys.modules["antenv.axon_hooks"] = mod
    try:
        import antenv
        antenv.axon_hooks = mod
    except ImportError:
        pass

    lib = ctypes.CDLL(so_path)
    if not hasattr(lib, "axon_start_nrt_profile"):
        return
    lib.axon_start_nrt_profile.argtypes = [
        ctypes.POINTER(ctypes.c_int64), ctypes.c_size_t]
    lib.axon_start_nrt_profile.restype = ctypes.c_int64
    lib.axon_stop_nrt_profile.argtypes = [ctypes.c_char_p]
    lib.axon_stop_nrt_profile.restype = ctypes.c_int64

    @contextlib.contextmanager
    def _hook(output_dir, device_ids):
        import jax
        jax.devices()
        if device_ids:
            ids = (ctypes.c_int64 * len(device_ids))(*device_ids)
            rc = lib.axon_start_nrt_profile(ids, len(device_ids))
        else:
            rc = lib.axon_start_nrt_profile(None, 0)
        if rc != 0:
            raise RuntimeError(f"axon_start_nrt_profile rc={rc}")
        try:
            yield
        finally:
            n = lib.axon_stop_nrt_profile(str(output_dir).encode())
            print(f"profile: {n} file(s) written to {output_dir}")

    mod.set_axon_ntff_profile_hook(_hook)


def run(x, weight, bias, dec_w, dec_b, trace=False):
    from concourse import bass_utils
    from concourse.bass_utils import run_bass_kernel_spmd

    if trace:
        _install_ntff_hook()
        # artifact upload needs a bucket that doesn't exist here
        bass_utils.upload_artifacts = lambda tmpdir: tmpdir

    nc = _get_nc()
    in_maps = make_in_maps(x, weight, bias, dec_w)
    r = run_bass_kernel_spmd(nc, in_maps, list(range(NCORES)), trace=trace)
    return combine(r.results, dec_b), r


def kernel(x, weight, bias, dec_w, dec_b):
    out, _ = run(x, weight, bias, dec_w, dec_b, trace=False)
    return out



# revision 2
# speedup vs baseline: 1.1703x; 1.1703x over previous
"""Trainium2 Bass kernel for nn_LCN (locally-connected network).

Computation:
  x: (512, 1, 280, 280) -> non-overlapping 28x28 patches (10x10 grid, P=100)
  y[b, f, p] = sum_{k,l} x[b, 28ph+k, 28pw+l] * w[f*100+p, 0, k, l]
  y = relu(y + bias[f*100+p]);  out = y_flat @ dec_w.T + dec_b   (j = f*100 + p)

Strategy (v2): the problem is HBM-stream-bound, so minimize bytes moved and
keep the device program trivial:
  - Host casts x to bf16 (error budget 2e-2 >> bf16's ~5e-3; PE computed in
    bf16 anyway) -> halves the dominant x stream vs fp32.
  - Host pre-transposes x into contraction-major chunk layout
    [112=(kk,l), (patch, t, batch)] so the PE does NO transposes and no
    DVE/ACT im2col: just the real matmuls, straight off the DMA stream.
  - Sharding: 2 batch halves (256 images) x 4 patch quarters (25 patches).
    Per core: x 10.0 MB + w 0.63 MB ~ 10.7 MB -> ~30us at ~358 GB/s/NC.
  - Mains: per patch 7 accumulating bf16 matmuls lhsT=w[112,16],
    rhs=x[112,256] -> y PSUM, 4 patches per PSUM tile at col offsets
    0/32/64/96 (tile_position -> concurrent col-tile streams).
  - ACT: relu(y + bias) -> y_sb bf16; decoder matmul per group deferred by
    one group so the relu never stalls the PE; accumulated in PSUM.
Host sums the 4 patch-quarter partial decoder outputs and adds dec_b.
"""

import sys

import numpy as np

for _p in ("/opt/trn_rl_repo", "/opt/trn_rl_repo/concourse"):
    if _p not in sys.path:
        sys.path.insert(0, _p)

import concourse.bass as bass
import concourse.mybir as mybir
import concourse.tile as tile
from concourse import bacc

F32 = mybir.dt.float32
BF16 = mybir.dt.bfloat16

# Problem constants
B, H, W = 512, 280, 280
KS = 28
HS = WS = 10
P = 100         # patches per image
F = 16
OUT = 10
NCORES = 8

# Sharding: 2 batch halves x 4 patch quarters
NB = 256        # images per core
NP = 25         # patches per core
NCHUNK = 7      # 112-pixel chunks per patch (4 k-rows x 28 cols each)
CK = 112        # contraction chunk size
NG = (NP + 3) // 4  # PSUM groups of 4 patches (last group: 1 patch)
YB = 3          # y PSUM bufs


def build_program():
    nc = bacc.Bacc("TRN2")
    x_d = nc.dram_tensor("x", [CK, NP * NCHUNK * NB], BF16, kind="ExternalInput")
    w_d = nc.dram_tensor("w", [CK, NP * NCHUNK * F], BF16, kind="ExternalInput")
    b_d = nc.dram_tensor("bias", [128, NG], F32, kind="ExternalInput")
    d_d = nc.dram_tensor("dec", [128, NG * OUT], BF16, kind="ExternalInput")
    o_d = nc.dram_tensor("out", [OUT, NB], F32, kind="ExternalOutput")

    with tile.TileContext(nc) as tc:
        with (
            tc.tile_pool(name="const", bufs=1) as constp,
            tc.tile_pool(name="xs", bufs=1) as xsp,
            tc.tile_pool(name="yps", bufs=YB, space="PSUM") as ypsp,
            tc.tile_pool(name="ops", bufs=1, space="PSUM") as opsp,
        ):
            w_sb = constp.tile([CK, NP * NCHUNK * F], BF16)
            bias_sb = constp.tile([128, NG], F32)
            dec_sb = constp.tile([128, NG * OUT], BF16)
            y_sb = constp.tile([128, NG * NB], BF16)
            zero_sb = constp.tile([128, NB], F32)
            out_sb = constp.tile([OUT, NB], F32)
            out_ps = opsp.tile([OUT, NB], F32)

            # w/bias/dec on the ACT HWDGE ring so they never sit in front of
            # x slices in the sync-ring FIFO
            nc.scalar.dma_start(out=w_sb[:], in_=w_d[:])
            nc.scalar.dma_start(out=bias_sb[:], in_=b_d[:])
            nc.scalar.dma_start(out=dec_sb[:], in_=d_d[:])
            nc.gpsimd.memset(zero_sb[:], 0.0)

            # x group slices: 6 x 1.6 MB + 1 x 0.4 MB, streamed in
            # consumption order on the sync ring
            x_tiles = []
            for g in range(NG):
                npg = min(4, NP - 4 * g)
                t = xsp.tile([CK, npg * NCHUNK * NB], BF16, name=f"xg{g}")
                off = 4 * g * NCHUNK * NB
                nc.sync.dma_start(out=t[:], in_=x_d[:, off:off + npg * NCHUNK * NB])
                x_tiles.append(t)

            def emit_dec(g):
                nc.tensor.matmul(
                    out_ps[:],
                    dec_sb[:, g * OUT:(g + 1) * OUT],
                    y_sb[:, g * NB:(g + 1) * NB],
                    start=(g == 0),
                    stop=(g == NG - 1),
                )

            for g in range(NG):
                npg = min(4, NP - 4 * g)
                yt = ypsp.tile([128, NB], F32, name="y_ps")
                if g < YB:
                    # clear stale/NaN PSUM so the gap rows are finite zeros
                    nc.vector.tensor_copy(yt[:], zero_sb[:])
                for t in range(NCHUNK):
                    for q in range(npg):
                        pl = 4 * g + q
                        nc.tensor.matmul(
                            yt[32 * q:32 * q + F, :],
                            w_sb[:, (pl * NCHUNK + t) * F:
                                 (pl * NCHUNK + t + 1) * F],
                            x_tiles[g][:, (q * NCHUNK + t) * NB:
                                       (q * NCHUNK + t + 1) * NB],
                            start=(t == 0),
                            stop=(t == NCHUNK - 1),
                            tile_position=(0, 32 * q),
                        )
                nc.scalar.activation(
                    out=y_sb[:, g * NB:(g + 1) * NB],
                    in_=yt[:],
                    func=mybir.ActivationFunctionType.Relu,
                    bias=bias_sb[:, g:g + 1],
                )
                # decoder deferred one group so its y_sb dependency (the
                # relu above) never stalls the PE behind independent mains
                if g > 0:
                    emit_dec(g - 1)
            emit_dec(NG - 1)

            nc.vector.tensor_copy(out_sb[:], out_ps[:])
            nc.sync.dma_start(out=o_d[:], in_=out_sb[:])

    return nc


def make_in_maps(x, weight, bias, dec_w):
    import ml_dtypes
    bf16 = ml_dtypes.bfloat16

    x = np.asarray(x, np.float32).reshape(B, H, W)
    xb = x.astype(bf16)
    # (bh, b, hs, t, kk, ws, l) -> (bh, kk, l, hs, ws, t, b)
    x7 = xb.reshape(2, NB, HS, NCHUNK, 4, WS, KS)
    xt = np.ascontiguousarray(x7.transpose(0, 4, 6, 2, 5, 3, 1))
    xt = xt.reshape(2, CK, P, NCHUNK * NB)

    # weight row j = f*100 + p; pixel k = t*4 + kk
    w5 = np.asarray(weight, np.float32).reshape(F, P, NCHUNK, 4, KS)
    wt = np.ascontiguousarray(w5.transpose(3, 4, 1, 2, 0)).astype(bf16)
    wt = wt.reshape(CK, P, NCHUNK * F)

    b2 = np.asarray(bias, np.float32).reshape(F, P)
    d3 = np.asarray(dec_w, np.float32).reshape(OUT, F, P)

    in_maps = []
    for core in range(NCORES):
        bh, q4 = core // 4, core % 4
        p0 = q4 * NP
        xs = np.ascontiguousarray(xt[bh, :, p0:p0 + NP]).reshape(
            CK, NP * NCHUNK * NB)
        ws = np.ascontiguousarray(wt[:, p0:p0 + NP]).reshape(
            CK, NP * NCHUNK * F)
        bst = np.zeros((128, NG), np.float32)
        dst = np.zeros((128, NG * OUT), np.float32)
        for pl in range(NP):
            G, q = pl // 4, pl % 4
            bst[32 * q:32 * q + F, G] = b2[:, p0 + pl]
            dst[32 * q:32 * q + F, G * OUT:(G + 1) * OUT] = d3[:, :, p0 + pl].T
        in_maps.append(
            {"x": xs, "w": ws, "bias": bst, "dec": dst.astype(bf16)})
    return in_maps


def combine(results, dec_b):
    dec_b = np.asarray(dec_b, np.float32)
    out = np.zeros((B, OUT), np.float32)
    for bh in range(2):
        part = results[4 * bh]["out"].astype(np.float32)
        for q4 in range(1, 4):
            part = part + results[4 * bh + q4]["out"]
        out[bh * NB:(bh + 1) * NB] = part.T + dec_b
    return out


_cache = {}


def _get_nc():
    if "nc" not in _cache:
        nc = build_program()
        nc.finalize()
        _cache["nc"] = nc
    return _cache["nc"]


def _install_ntff_hook():
    """Provide the missing antenv.axon_hooks module so trace=True works
    under axon (replicates trn_boot._ntff_profile_via_ctypes)."""
    import contextlib
    import ctypes
    import types

    if "antenv.axon_hooks" in sys.modules:
        return
    so_path = "/opt/axon/libaxon_pjrt.so"
    holder = {}
    mod = types.ModuleType("antenv.axon_hooks")
    mod.set_axon_ntff_profile_hook = lambda h: holder.__setitem__("h", h)
    mod.get_axon_ntff_profile_hook = lambda: holder.get("h")
    sys.modules["antenv.axon_hooks"] = mod
    try:
        import antenv
        antenv.axon_hooks = mod
    except ImportError:
        pass

    lib = ctypes.CDLL(so_path)
    if not hasattr(lib, "axon_start_nrt_profile"):
        return
    lib.axon_start_nrt_profile.argtypes = [
        ctypes.POINTER(ctypes.c_int64), ctypes.c_size_t]
    lib.axon_start_nrt_profile.restype = ctypes.c_int64
    lib.axon_stop_nrt_profile.argtypes = [ctypes.c_char_p]
    lib.axon_stop_nrt_profile.restype = ctypes.c_int64

    @contextlib.contextmanager
    def _hook(output_dir, device_ids):
        import jax
        jax.devices()
        if device_ids:
            ids = (ctypes.c_int64 * len(device_ids))(*device_ids)
            rc = lib.axon_start_nrt_profile(ids, len(device_ids))
        else:
            rc = lib.axon_start_nrt_profile(None, 0)
        if rc != 0:
            raise RuntimeError(f"axon_start_nrt_profile rc={rc}")
        try:
            yield
        finally:
            n = lib.axon_stop_nrt_profile(str(output_dir).encode())
            print(f"profile: {n} file(s) written to {output_dir}")

    mod.set_axon_ntff_profile_hook(_hook)


def run(x, weight, bias, dec_w, dec_b, trace=False):
    from concourse import bass_utils
    from concourse.bass_utils import run_bass_kernel_spmd

    if trace:
        _install_ntff_hook()
        # artifact upload needs a bucket that doesn't exist here
        bass_utils.upload_artifacts = lambda tmpdir: tmpdir

    nc = _get_nc()
    in_maps = make_in_maps(x, weight, bias, dec_w)
    r = run_bass_kernel_spmd(nc, in_maps, list(range(NCORES)), trace=trace)
    return combine(r.results, dec_b), r


def kernel(x, weight, bias, dec_w, dec_b):
    out, _ = run(x, weight, bias, dec_w, dec_b, trace=False)
    return out


# revision 7
# speedup vs baseline: 1.3840x; 1.1827x over previous
"""Trainium2 Bass kernel for nn_LCN (locally-connected network).

Computation:
  x: (512, 1, 280, 280) -> non-overlapping 28x28 patches (10x10 grid, P=100)
  y[b, f, p] = sum_{k,l} x[b, 28ph+k, 28pw+l] * w[f*100+p, 0, k, l]
  y = relu(y + bias[f*100+p]);  out = y_flat @ dec_w.T + dec_b   (j = f*100 + p)

Strategy (v2): the problem is HBM-stream-bound, so minimize bytes moved and
keep the device program trivial:
  - Host casts x to bf16 (error budget 2e-2 >> bf16's ~5e-3; PE computed in
    bf16 anyway) -> halves the dominant x stream vs fp32.
  - Host pre-transposes x into contraction-major chunk layout
    [112=(kk,l), (patch, t, batch)] so the PE does NO transposes and no
    DVE/ACT im2col: just the real matmuls, straight off the DMA stream.
  - Sharding: 2 batch halves (256 images) x 4 patch quarters (25 patches).
    Per core: x 10.0 MB + w 0.63 MB ~ 10.7 MB -> ~30us at ~358 GB/s/NC.
  - Mains: per patch 7 accumulating bf16 matmuls lhsT=w[112,16],
    rhs=x[112,256] -> y PSUM, 4 patches per PSUM tile at col offsets
    0/32/64/96 (tile_position -> concurrent col-tile streams).
  - ACT: relu(y + bias) -> y_sb bf16; decoder matmul per group deferred by
    one group so the relu never stalls the PE; accumulated in PSUM.
Host sums the 4 patch-quarter partial decoder outputs and adds dec_b.
"""

import sys

import numpy as np

for _p in ("/opt/trn_rl_repo", "/opt/trn_rl_repo/concourse"):
    if _p not in sys.path:
        sys.path.insert(0, _p)

import concourse.bass as bass
import concourse.mybir as mybir
import concourse.tile as tile
from concourse import bacc

F32 = mybir.dt.float32
BF16 = mybir.dt.bfloat16

# Problem constants
B, H, W = 512, 280, 280
KS = 28
HS = WS = 10
P = 100         # patches per image
F = 16
OUT = 10
NCORES = 8

# Sharding: 2 batch halves x 4 patch quarters
NB = 256        # images per core
NP = 25         # patches per core
NCHUNK = 7      # 112-pixel chunks per patch (4 k-rows x 28 cols each)
CK = 112        # contraction chunk size
NG = (NP + 3) // 4  # PSUM groups of 4 patches (last group: 1 patch)
YB = 3          # y PSUM bufs


def build_program():
    nc = bacc.Bacc("TRN2")
    # x/w hold bf16 data but are DMA'd as f32 (same bytes): bf16-typed DMA
    # descriptors run at 16B/cycle (half the 32B AXI beat) -- measured 13.5
    # vs 25 GB/s per engine. The PE reads them through a bf16 bitcast.
    x_d = nc.dram_tensor("x", [CK, NP * NCHUNK * NB // 2], F32, kind="ExternalInput")
    w_d = nc.dram_tensor("w", [CK, NP * NCHUNK * F // 2], F32, kind="ExternalInput")
    b_d = nc.dram_tensor("bias", [128, NG], F32, kind="ExternalInput")
    d_d = nc.dram_tensor("dec", [128, NG * OUT], BF16, kind="ExternalInput")
    o_d = nc.dram_tensor("out", [OUT, NB], F32, kind="ExternalOutput")

    with tile.TileContext(nc) as tc:
        with (
            tc.tile_pool(name="const", bufs=1) as constp,
            tc.tile_pool(name="xs", bufs=1) as xsp,
            tc.tile_pool(name="yps", bufs=YB, space="PSUM") as ypsp,
            tc.tile_pool(name="ops", bufs=1, space="PSUM") as opsp,
        ):
            w_sb = constp.tile([CK, NP * NCHUNK * F // 2], F32)
            bias_sb = constp.tile([128, NG], F32)
            dec_sb = constp.tile([128, NG * OUT], BF16)
            y_sb = constp.tile([128, NG * NB], BF16)
            zero_sb = constp.tile([128, NB], F32)
            out_sb = constp.tile([OUT, NB], F32)
            out_ps = opsp.tile([OUT, NB], F32)

            # w/bias/dec on the ACT HWDGE ring so they never sit in front of
            # x slices in the sync-ring FIFO
            nc.scalar.dma_start(out=w_sb[:], in_=w_d[:])
            nc.scalar.dma_start(out=bias_sb[:], in_=b_d[:])
            nc.scalar.dma_start(out=dec_sb[:], in_=d_d[:])
            nc.gpsimd.memset(zero_sb[:], 0.0)

            # x group slices: 6 x 1.6 MB + 1 x 0.4 MB, streamed in
            # consumption order on the sync ring
            x_tiles = []
            for g in range(NG):
                npg = min(4, NP - 4 * g)
                t = xsp.tile([CK, npg * NCHUNK * NB // 2], F32, name=f"xg{g}")
                off = 4 * g * NCHUNK * NB // 2
                nc.sync.dma_start(
                    out=t[:], in_=x_d[:, off:off + npg * NCHUNK * NB // 2])
                x_tiles.append(t)
            w_bf = w_sb[:].bitcast(BF16)

            def emit_dec(g):
                nc.tensor.matmul(
                    out_ps[:],
                    dec_sb[:, g * OUT:(g + 1) * OUT],
                    y_sb[:, g * NB:(g + 1) * NB],
                    start=(g == 0),
                    stop=(g == NG - 1),
                )

            for g in range(NG):
                npg = min(4, NP - 4 * g)
                yt = ypsp.tile([128, NB], F32, name="y_ps")
                if g < YB:
                    # clear stale/NaN PSUM so the gap rows are finite zeros
                    nc.vector.tensor_copy(yt[:], zero_sb[:])
                xg_bf = x_tiles[g][:].bitcast(BF16)
                for t in range(NCHUNK):
                    for q in range(npg):
                        pl = 4 * g + q
                        nc.tensor.matmul(
                            yt[32 * q:32 * q + F, :],
                            w_bf[:, (pl * NCHUNK + t) * F:
                                 (pl * NCHUNK + t + 1) * F],
                            xg_bf[:, (q * NCHUNK + t) * NB:
                                  (q * NCHUNK + t + 1) * NB],
                            start=(t == 0),
                            stop=(t == NCHUNK - 1),
                            tile_position=(0, 32 * q),
                        )
                nc.scalar.activation(
                    out=y_sb[:, g * NB:(g + 1) * NB],
                    in_=yt[:],
                    func=mybir.ActivationFunctionType.Relu,
                    bias=bias_sb[:, g:g + 1],
                )
                # decoder deferred one group so its y_sb dependency (the
                # relu above) never stalls the PE behind independent mains
                if g > 0:
                    emit_dec(g - 1)
            emit_dec(NG - 1)

            nc.vector.tensor_copy(out_sb[:], out_ps[:])
            nc.sync.dma_start(out=o_d[:], in_=out_sb[:])

    return nc


def make_in_maps(x, weight, bias, dec_w):
    import ml_dtypes
    bf16 = ml_dtypes.bfloat16

    x = np.asarray(x, np.float32).reshape(B, H, W)
    xb = x.astype(bf16)
    # (bh, b, hs, t, kk, ws, l) -> (bh, kk, l, hs, ws, t, b)
    x7 = xb.reshape(2, NB, HS, NCHUNK, 4, WS, KS)
    xt = np.ascontiguousarray(x7.transpose(0, 4, 6, 2, 5, 3, 1))
    xt = xt.reshape(2, CK, P, NCHUNK * NB)

    # weight row j = f*100 + p; pixel k = t*4 + kk
    w5 = np.asarray(weight, np.float32).reshape(F, P, NCHUNK, 4, KS)
    wt = np.ascontiguousarray(w5.transpose(3, 4, 1, 2, 0)).astype(bf16)
    wt = wt.reshape(CK, P, NCHUNK * F)

    b2 = np.asarray(bias, np.float32).reshape(F, P)
    d3 = np.asarray(dec_w, np.float32).reshape(OUT, F, P)

    in_maps = []
    for core in range(NCORES):
        bh, q4 = core // 4, core % 4
        p0 = q4 * NP
        xs = np.ascontiguousarray(xt[bh, :, p0:p0 + NP]).reshape(
            CK, NP * NCHUNK * NB)
        ws = np.ascontiguousarray(wt[:, p0:p0 + NP]).reshape(
            CK, NP * NCHUNK * F)
        bst = np.zeros((128, NG), np.float32)
        dst = np.zeros((128, NG * OUT), np.float32)
        for pl in range(NP):
            G, q = pl // 4, pl % 4
            bst[32 * q:32 * q + F, G] = b2[:, p0 + pl]
            dst[32 * q:32 * q + F, G * OUT:(G + 1) * OUT] = d3[:, :, p0 + pl].T
        in_maps.append(
            {"x": xs.view(np.float32), "w": ws.view(np.float32),
             "bias": bst, "dec": dst.astype(bf16)})
    return in_maps


def combine(results, dec_b):
    dec_b = np.asarray(dec_b, np.float32)
    out = np.zeros((B, OUT), np.float32)
    for bh in range(2):
        part = results[4 * bh]["out"].astype(np.float32)
        for q4 in range(1, 4):
            part = part + results[4 * bh + q4]["out"]
        out[bh * NB:(bh + 1) * NB] = part.T + dec_b
    return out


_cache = {}


def _get_nc():
    if "nc" not in _cache:
        nc = build_program()
        nc.finalize()
        _cache["nc"] = nc
    return _cache["nc"]


def _install_ntff_hook():
    """Provide the missing antenv.axon_hooks module so trace=True works
    under axon (replicates trn_boot._ntff_profile_via_ctypes)."""
    import contextlib
    import ctypes
    import types

    if "antenv.axon_hooks" in sys.modules:
        return
    so_path = "/opt/axon/libaxon_pjrt.so"
    holder = {}
    mod = types.ModuleType("antenv.axon_hooks")
    mod.set_axon_ntff_profile_hook = lambda h: holder.__setitem__("h", h)
    mod.get_axon_ntff_profile_hook = lambda: holder.get("h")
    sys.modules["antenv.axon_hooks"] = mod
    try:
        import antenv
        antenv.axon_hooks = mod
    except ImportError:
        pass

    lib = ctypes.CDLL(so_path)
    if not hasattr(lib, "axon_start_nrt_profile"):
        return
    lib.axon_start_nrt_profile.argtypes = [
        ctypes.POINTER(ctypes.c_int64), ctypes.c_size_t]
    lib.axon_start_nrt_profile.restype = ctypes.c_int64
    lib.axon_stop_nrt_profile.argtypes = [ctypes.c_char_p]
    lib.axon_stop_nrt_profile.restype = ctypes.c_int64

    @contextlib.contextmanager
    def _hook(output_dir, device_ids):
        import jax
        jax.devices()
        if device_ids:
            ids = (ctypes.c_int64 * len(device_ids))(*device_ids)
            rc = lib.axon_start_nrt_profile(ids, len(device_ids))
        else:
            rc = lib.axon_start_nrt_profile(None, 0)
        if rc != 0:
            raise RuntimeError(f"axon_start_nrt_profile rc={rc}")
        try:
            yield
        finally:
            n = lib.axon_stop_nrt_profile(str(output_dir).encode())
            print(f"profile: {n} file(s) written to {output_dir}")

    mod.set_axon_ntff_profile_hook(_hook)


def run(x, weight, bias, dec_w, dec_b, trace=False):
    from concourse import bass_utils
    from concourse.bass_utils import run_bass_kernel_spmd

    if trace:
        _install_ntff_hook()
        # artifact upload needs a bucket that doesn't exist here
        bass_utils.upload_artifacts = lambda tmpdir: tmpdir

    nc = _get_nc()
    in_maps = make_in_maps(x, weight, bias, dec_w)
    r = run_bass_kernel_spmd(nc, in_maps, list(range(NCORES)), trace=trace)
    return combine(r.results, dec_b), r


def kernel(x, weight, bias, dec_w, dec_b):
    out, _ = run(x, weight, bias, dec_w, dec_b, trace=False)
    return out


# revision 12
# speedup vs baseline: 1.6027x; 1.1580x over previous
"""Trainium2 Bass kernel for nn_LCN (locally-connected network).

Computation:
  x: (512, 1, 280, 280) -> non-overlapping 28x28 patches (10x10 grid, P=100)
  y[b, f, p] = sum_{k,l} x[b, 28ph+k, 28pw+l] * w[f*100+p, 0, k, l]
  y = relu(y + bias[f*100+p]);  out = y_flat @ dec_w.T + dec_b   (j = f*100 + p)

Strategy (v2): the problem is HBM-stream-bound, so minimize bytes moved and
keep the device program trivial:
  - Host casts x to bf16 (error budget 2e-2 >> bf16's ~5e-3; PE computed in
    bf16 anyway) -> halves the dominant x stream vs fp32.
  - Host pre-transposes x into contraction-major chunk layout
    [112=(kk,l), (patch, t, batch)] so the PE does NO transposes and no
    DVE/ACT im2col: just the real matmuls, straight off the DMA stream.
  - Sharding: 2 batch halves (256 images) x 4 patch quarters (25 patches).
    Per core: x 10.0 MB + w 0.63 MB ~ 10.7 MB -> ~30us at ~358 GB/s/NC.
  - Mains: per patch 7 accumulating bf16 matmuls lhsT=w[112,16],
    rhs=x[112,256] -> y PSUM, 4 patches per PSUM tile at col offsets
    0/32/64/96 (tile_position -> concurrent col-tile streams).
  - ACT: relu(y + bias) -> y_sb bf16; decoder matmul per group deferred by
    one group so the relu never stalls the PE; accumulated in PSUM.
Host sums the 4 patch-quarter partial decoder outputs and adds dec_b.
"""

import sys

import numpy as np

for _p in ("/opt/trn_rl_repo", "/opt/trn_rl_repo/concourse"):
    if _p not in sys.path:
        sys.path.insert(0, _p)

import concourse.bass as bass
import concourse.mybir as mybir
import concourse.tile as tile
from concourse import bacc

F32 = mybir.dt.float32
BF16 = mybir.dt.bfloat16

# Problem constants
B, H, W = 512, 280, 280
KS = 28
HS = WS = 10
P = 100         # patches per image
F = 16
OUT = 10
NCORES = 8

# Sharding: 2 batch halves x 4 patch quarters
NB = 256        # images per core
NP = 25         # patches per core
NCHUNK = 7      # 112-pixel chunks per patch (4 k-rows x 28 cols each)
CK = 112        # contraction chunk size
NG = (NP + 3) // 4  # PSUM groups of 4 patches (last group: 1 patch)
YB = 3          # y PSUM bufs


def build_program():
    nc = bacc.Bacc("TRN2")
    # x/w hold bf16 data but are DMA'd as f32 (same bytes): bf16-typed DMA
    # descriptors run at 16B/cycle (half the 32B AXI beat) -- measured 13.5
    # vs 25 GB/s per engine. The PE reads them through a bf16 bitcast.
    # x is padded to 128 partition rows (112 real + 16 junk): a 112-row DMA
    # splits 7 rows/engine across 8-partition AXI ports, so engines contend
    # pairwise on ports; 128 rows align engine<->port for full rate.
    x_d = nc.dram_tensor("x", [128, NP * NCHUNK * NB // 2], F32, kind="ExternalInput")
    w_d = nc.dram_tensor("w", [CK, NP * NCHUNK * F // 2], F32, kind="ExternalInput")
    b_d = nc.dram_tensor("bias", [128, NG], F32, kind="ExternalInput")
    d_d = nc.dram_tensor("dec", [128, NG * OUT], BF16, kind="ExternalInput")
    o_d = nc.dram_tensor("out", [OUT, NB], F32, kind="ExternalOutput")

    with tile.TileContext(nc) as tc:
        with (
            tc.tile_pool(name="const", bufs=1) as constp,
            tc.tile_pool(name="xs", bufs=1) as xsp,
            tc.tile_pool(name="yps", bufs=YB, space="PSUM") as ypsp,
            tc.tile_pool(name="ops", bufs=1, space="PSUM") as opsp,
        ):
            w_sb = constp.tile([CK, NP * NCHUNK * F // 2], F32)
            bias_sb = constp.tile([128, NG], F32)
            dec_sb = constp.tile([128, NG * OUT], BF16)
            y_sb = constp.tile([128, NG * NB], BF16)
            zero_sb = constp.tile([128, NB], F32)
            out_sb = constp.tile([OUT, NB], F32)
            out_ps = opsp.tile([OUT, NB], F32)

            # w/bias/dec on the ACT HWDGE ring so they never sit in front of
            # x slices in the sync-ring FIFO
            nc.scalar.dma_start(out=w_sb[:], in_=w_d[:])
            nc.scalar.dma_start(out=bias_sb[:], in_=b_d[:])
            nc.scalar.dma_start(out=dec_sb[:], in_=d_d[:])
            nc.gpsimd.memset(zero_sb[:], 0.0)

            # x group slices: 6 x 1.6 MB + 1 x 0.4 MB, streamed in
            # consumption order on the sync ring
            x_tiles = []
            for g in range(NG):
                npg = min(4, NP - 4 * g)
                t = xsp.tile([128, npg * NCHUNK * NB // 2], F32, name=f"xg{g}")
                off = 4 * g * NCHUNK * NB // 2
                nc.sync.dma_start(
                    out=t[:], in_=x_d[:, off:off + npg * NCHUNK * NB // 2])
                x_tiles.append(t)
            w_bf = w_sb[:].bitcast(BF16)

            def emit_dec(g):
                nc.tensor.matmul(
                    out_ps[:],
                    dec_sb[:, g * OUT:(g + 1) * OUT],
                    y_sb[:, g * NB:(g + 1) * NB],
                    start=(g == 0),
                    stop=(g == NG - 1),
                )

            for g in range(NG):
                npg = min(4, NP - 4 * g)
                yt = ypsp.tile([128, NB], F32, name="y_ps")
                if g < YB:
                    # clear stale/NaN PSUM so the gap rows are finite zeros
                    nc.vector.tensor_copy(yt[:], zero_sb[:])
                xg_bf = x_tiles[g][:].bitcast(BF16)[0:CK, :]
                for t in range(NCHUNK):
                    for q in range(npg):
                        pl = 4 * g + q
                        nc.tensor.matmul(
                            yt[32 * q:32 * q + F, :],
                            w_bf[:, (pl * NCHUNK + t) * F:
                                 (pl * NCHUNK + t + 1) * F],
                            xg_bf[:, (q * NCHUNK + t) * NB:
                                  (q * NCHUNK + t + 1) * NB],
                            start=(t == 0),
                            stop=(t == NCHUNK - 1),
                            tile_position=(0, 32 * q),
                        )
                nc.scalar.activation(
                    out=y_sb[:, g * NB:(g + 1) * NB],
                    in_=yt[:],
                    func=mybir.ActivationFunctionType.Relu,
                    bias=bias_sb[:, g:g + 1],
                )
                # decoder deferred one group so its y_sb dependency (the
                # relu above) never stalls the PE behind independent mains
                if g > 0:
                    emit_dec(g - 1)
            emit_dec(NG - 1)

            nc.vector.tensor_copy(out_sb[:], out_ps[:])
            nc.sync.dma_start(out=o_d[:], in_=out_sb[:])

    return nc


def make_in_maps(x, weight, bias, dec_w):
    import ml_dtypes
    bf16 = ml_dtypes.bfloat16

    x = np.asarray(x, np.float32).reshape(B, H, W)
    xb = x.astype(bf16)
    # (bh, b, hs, t, kk, ws, l) -> (bh, kk, l, hs, ws, t, b)
    x7 = xb.reshape(2, NB, HS, NCHUNK, 4, WS, KS)
    xt = np.zeros((2, 128, P, NCHUNK * NB), bf16)
    xt[:, :CK] = np.ascontiguousarray(x7.transpose(0, 4, 6, 2, 5, 3, 1)).reshape(
        2, CK, P, NCHUNK * NB)

    # weight row j = f*100 + p; pixel k = t*4 + kk
    w5 = np.asarray(weight, np.float32).reshape(F, P, NCHUNK, 4, KS)
    wt = np.ascontiguousarray(w5.transpose(3, 4, 1, 2, 0)).astype(bf16)
    wt = wt.reshape(CK, P, NCHUNK * F)

    b2 = np.asarray(bias, np.float32).reshape(F, P)
    d3 = np.asarray(dec_w, np.float32).reshape(OUT, F, P)

    in_maps = []
    for core in range(NCORES):
        bh, q4 = core // 4, core % 4
        p0 = q4 * NP
        xs = np.ascontiguousarray(xt[bh, :, p0:p0 + NP]).reshape(
            128, NP * NCHUNK * NB)
        ws = np.ascontiguousarray(wt[:, p0:p0 + NP]).reshape(
            CK, NP * NCHUNK * F)
        bst = np.zeros((128, NG), np.float32)
        dst = np.zeros((128, NG * OUT), np.float32)
        for pl in range(NP):
            G, q = pl // 4, pl % 4
            bst[32 * q:32 * q + F, G] = b2[:, p0 + pl]
            dst[32 * q:32 * q + F, G * OUT:(G + 1) * OUT] = d3[:, :, p0 + pl].T
        in_maps.append(
            {"x": xs.view(np.float32), "w": ws.view(np.float32),
             "bias": bst, "dec": dst.astype(bf16)})
    return in_maps


def combine(results, dec_b):
    dec_b = np.asarray(dec_b, np.float32)
    out = np.zeros((B, OUT), np.float32)
    for bh in range(2):
        part = results[4 * bh]["out"].astype(np.float32)
        for q4 in range(1, 4):
            part = part + results[4 * bh + q4]["out"]
        out[bh * NB:(bh + 1) * NB] = part.T + dec_b
    return out


_cache = {}


def _get_nc():
    if "nc" not in _cache:
        nc = build_program()
        nc.finalize()
        _cache["nc"] = nc
    return _cache["nc"]


def _install_ntff_hook():
    """Provide the missing antenv.axon_hooks module so trace=True works
    under axon (replicates trn_boot._ntff_profile_via_ctypes)."""
    import contextlib
    import ctypes
    import types

    if "antenv.axon_hooks" in sys.modules:
        return
    so_path = "/opt/axon/libaxon_pjrt.so"
    holder = {}
    mod = types.ModuleType("antenv.axon_hooks")
    mod.set_axon_ntff_profile_hook = lambda h: holder.__setitem__("h", h)
    mod.get_axon_ntff_profile_hook = lambda: holder.get("h")
    sys.modules["antenv.axon_hooks"] = mod
    try:
        import antenv
        antenv.axon_hooks = mod
    except ImportError:
        pass

    lib = ctypes.CDLL(so_path)
    if not hasattr(lib, "axon_start_nrt_profile"):
        return
    lib.axon_start_nrt_profile.argtypes = [
        ctypes.POINTER(ctypes.c_int64), ctypes.c_size_t]
    lib.axon_start_nrt_profile.restype = ctypes.c_int64
    lib.axon_stop_nrt_profile.argtypes = [ctypes.c_char_p]
    lib.axon_stop_nrt_profile.restype = ctypes.c_int64

    @contextlib.contextmanager
    def _hook(output_dir, device_ids):
        import jax
        jax.devices()
        if device_ids:
            ids = (ctypes.c_int64 * len(device_ids))(*device_ids)
            rc = lib.axon_start_nrt_profile(ids, len(device_ids))
        else:
            rc = lib.axon_start_nrt_profile(None, 0)
        if rc != 0:
            raise RuntimeError(f"axon_start_nrt_profile rc={rc}")
        try:
            yield
        finally:
            n = lib.axon_stop_nrt_profile(str(output_dir).encode())
            print(f"profile: {n} file(s) written to {output_dir}")

    mod.set_axon_ntff_profile_hook(_hook)


def run(x, weight, bias, dec_w, dec_b, trace=False):
    from concourse import bass_utils
    from concourse.bass_utils import run_bass_kernel_spmd

    if trace:
        _install_ntff_hook()
        # artifact upload needs a bucket that doesn't exist here
        bass_utils.upload_artifacts = lambda tmpdir: tmpdir

    nc = _get_nc()
    in_maps = make_in_maps(x, weight, bias, dec_w)
    r = run_bass_kernel_spmd(nc, in_maps, list(range(NCORES)), trace=trace)
    return combine(r.results, dec_b), r


def kernel(x, weight, bias, dec_w, dec_b):
    out, _ = run(x, weight, bias, dec_w, dec_b, trace=False)
    return out


# revision 16
# speedup vs baseline: 1.6512x; 1.0303x over previous
"""Trainium2 Bass kernel for nn_LCN (locally-connected network).

Computation:
  x: (512, 1, 280, 280) -> non-overlapping 28x28 patches (10x10 grid, P=100)
  y[b, f, p] = sum_{k,l} x[b, 28ph+k, 28pw+l] * w[f*100+p, 0, k, l]
  y = relu(y + bias[f*100+p]);  out = y_flat @ dec_w.T + dec_b   (j = f*100 + p)

Strategy (v3): the problem is HBM-stream-bound, so minimize bytes moved and
keep the device program trivial:
  - Host casts x to bf16 (error budget 2e-2 >> bf16's ~5e-3; PE computed in
    bf16 anyway) -> halves the dominant x stream vs fp32.
  - Host pre-transposes x into contraction-major chunk layout so the PE
    does NO transposes / im2col: just matmuls, straight off the DMA stream.
  - DMA details that matter (measured): bf16-typed DMA runs at half the
    AXI beat width, so all streams are f32-typed and bitcast to bf16 for
    the PE; and only 128-partition-row DMAs align the 16 SDMA engines 1:1
    with the 16 SBUF AXI ports (112 rows -> pairwise port contention,
    16 vs 25 GB/s/engine). Hence each patch's 784 pixels are split as
    6 full K=128 chunks + a 16-pixel tail; tails of 4 patches pack into
    the 4 32-row strips of one column block and are applied by diagonal
    row+col-tiled K=16 matmuls.
  - Sharding: 2 batch halves (256 images) x 4 patch quarters (25 patches).
    Per core: x 9.8+0.5 MB + w 0.7 MB -> ~32us at ~343 GB/s/NC measured.
  - Mains: per patch 6 accumulating bf16 matmuls lhsT=w[128,16],
    rhs=x[128,256] -> y PSUM, 4 patches per PSUM tile at col offsets
    0/32/64/96 (tile_position -> concurrent col-tile streams), + 1 tail
    matmul lhsT=wt[16,16] at tile_position (32q,32q).
  - ACT: relu(y + bias) -> y_sb bf16; decoder matmul per group deferred by
    one group so the relu never stalls the PE; accumulated in PSUM.
  - The 1-patch group is processed first so the last DMA slice finishes
    into a wide 4-way-concurrent matmul tail.
Host sums the 4 patch-quarter partial decoder outputs and adds dec_b.
"""

import sys

import numpy as np

for _p in ("/opt/trn_rl_repo", "/opt/trn_rl_repo/concourse"):
    if _p not in sys.path:
        sys.path.insert(0, _p)

import concourse.bass as bass
import concourse.mybir as mybir
import concourse.tile as tile
from concourse import bacc

F32 = mybir.dt.float32
BF16 = mybir.dt.bfloat16

# Problem constants
B, H, W = 512, 280, 280
KS = 28
HS = WS = 10
P = 100         # patches per image
F = 16
OUT = 10
NCORES = 8

# Sharding: 2 batch halves x 4 patch quarters
NB = 256        # images per core
NP = 25         # patches per core
TM = 6          # full K=128 chunks per patch (768 of 784 pixels)
TAIL = 16       # leftover pixels per patch (pixel 768..783)
NG = (NP + 3) // 4  # PSUM groups of 4 patches (last group: 1 patch)
YB = 3          # y PSUM bufs


def build_program():
    nc = bacc.Bacc("TRN2")
    # All streams are f32-typed (bf16 data, bitcast on the PE side) and use
    # 128 partition rows -- both required for full DMA rate (see docstring).
    x_d = nc.dram_tensor("x", [128, NP * TM * NB // 2], F32, kind="ExternalInput")
    xt_d = nc.dram_tensor("xt", [128, NG * NB // 2], F32, kind="ExternalInput")
    w_d = nc.dram_tensor("w", [128, NP * TM * F // 2], F32, kind="ExternalInput")
    wt_d = nc.dram_tensor("wt", [128, NP * F // 2], F32, kind="ExternalInput")
    b_d = nc.dram_tensor("bias", [128, NG], F32, kind="ExternalInput")
    d_d = nc.dram_tensor("dec", [128, NG * OUT], BF16, kind="ExternalInput")
    o_d = nc.dram_tensor("out", [OUT, NB], F32, kind="ExternalOutput")

    with tile.TileContext(nc) as tc:
        with (
            tc.tile_pool(name="const", bufs=1) as constp,
            tc.tile_pool(name="xs", bufs=1) as xsp,
            tc.tile_pool(name="yps", bufs=YB, space="PSUM") as ypsp,
            tc.tile_pool(name="ops", bufs=1, space="PSUM") as opsp,
        ):
            w_sb = constp.tile([128, NP * TM * F // 2], F32)
            wt_sb = constp.tile([128, NP * F // 2], F32)
            bias_sb = constp.tile([128, NG], F32)
            dec_sb = constp.tile([128, NG * OUT], BF16)
            y_sb = constp.tile([128, NG * NB], BF16)
            zero_sb = constp.tile([128, NB], F32)
            out_sb = constp.tile([OUT, NB], F32)
            out_ps = opsp.tile([OUT, NB], F32)

            # w/bias/dec on the ACT HWDGE ring so they never sit in front of
            # x slices in the sync-ring FIFO
            nc.scalar.dma_start(out=w_sb[:], in_=w_d[:])
            nc.scalar.dma_start(out=wt_sb[:], in_=wt_d[:])
            nc.scalar.dma_start(out=bias_sb[:], in_=b_d[:])
            nc.scalar.dma_start(out=dec_sb[:], in_=d_d[:])
            nc.gpsimd.memset(zero_sb[:], 0.0)

            # groups processed 1-patch group first, so the final DMA slice
            # drains into a wide concurrent matmul tail
            order = [NG - 1] + list(range(NG - 1))

            # x tail block first (small, needed by every group), then main
            # slices on the sync ring in consumption order
            xt_sb = xsp.tile([128, NG * NB // 2], F32, name="xtail")
            nc.sync.dma_start(out=xt_sb[:], in_=xt_d[:])
            x_tiles = {}
            for g in order:
                npg = min(4, NP - 4 * g)
                t = xsp.tile([128, npg * TM * NB // 2], F32, name=f"xg{g}")
                off = 4 * g * TM * NB // 2
                nc.sync.dma_start(
                    out=t[:], in_=x_d[:, off:off + npg * TM * NB // 2])
                x_tiles[g] = t
            w_bf = w_sb[:].bitcast(BF16)
            wt_bf = wt_sb[:].bitcast(BF16)
            xt_bf = xt_sb[:].bitcast(BF16)

            def emit_dec(g, first, last):
                nc.tensor.matmul(
                    out_ps[:],
                    dec_sb[:, g * OUT:(g + 1) * OUT],
                    y_sb[:, g * NB:(g + 1) * NB],
                    start=first,
                    stop=last,
                )

            prev_g = None
            for idx, g in enumerate(order):
                npg = min(4, NP - 4 * g)
                yt = ypsp.tile([128, NB], F32, name="y_ps")
                if idx < YB:
                    # clear stale/NaN PSUM so the gap rows are finite zeros
                    nc.vector.tensor_copy(yt[:], zero_sb[:])
                xg_bf = x_tiles[g][:].bitcast(BF16)
                for t in range(TM):
                    for q in range(npg):
                        pl = 4 * g + q
                        nc.tensor.matmul(
                            yt[32 * q:32 * q + F, :],
                            w_bf[:, (pl * TM + t) * F:(pl * TM + t + 1) * F],
                            xg_bf[:, (q * TM + t) * NB:(q * TM + t + 1) * NB],
                            start=(t == 0),
                            stop=False,
                            tile_position=(0, 32 * q),
                        )
                for q in range(npg):
                    # 16-pixel tail: diagonal row+col tile (32q, 32q)
                    pl = 4 * g + q
                    blk = pl // 4
                    nc.tensor.matmul(
                        yt[32 * q:32 * q + F, :],
                        wt_bf[32 * q:32 * q + TAIL, pl * F:(pl + 1) * F],
                        xt_bf[32 * q:32 * q + TAIL, blk * NB:(blk + 1) * NB],
                        start=False,
                        stop=True,
                        tile_position=(32 * q, 32 * q),
                    )
                nc.scalar.activation(
                    out=y_sb[:, g * NB:(g + 1) * NB],
                    in_=yt[:],
                    func=mybir.ActivationFunctionType.Relu,
                    bias=bias_sb[:, g:g + 1],
                )
                # decoder deferred one group so its y_sb dependency (the
                # relu above) never stalls the PE behind independent mains
                if prev_g is not None:
                    emit_dec(prev_g, prev_g == order[0], False)
                prev_g = g
            emit_dec(prev_g, False, True)

            nc.vector.tensor_copy(out_sb[:], out_ps[:])
            nc.sync.dma_start(out=o_d[:], in_=out_sb[:])

    return nc


def make_in_maps(x, weight, bias, dec_w):
    import ml_dtypes
    bf16 = ml_dtypes.bfloat16

    x = np.asarray(x, np.float32).reshape(B, H, W)
    xb = x.astype(bf16)
    # (bh, b, hs, k, ws, l) -> (bh, hs, ws, k, l, b): per-patch pixel-major
    x6 = xb.reshape(2, NB, HS, KS, WS, KS)
    pp = np.ascontiguousarray(x6.transpose(0, 2, 4, 3, 5, 1)).reshape(
        2, P, KS * KS, NB)
    # main chunks: pixel j in [0,768) -> row j%128, col (p, t=j//128, b)
    xm = np.ascontiguousarray(
        pp[:, :, :TM * 128].reshape(2, P, TM, 128, NB).transpose(0, 3, 1, 2, 4))
    tl = pp[:, :, TM * 128:]                      # (2, P, 16, NB)

    # weight row j = f*100 + p (pixel-major k*28+l)
    w3 = np.asarray(weight, np.float32).reshape(F, P, KS * KS)
    wm = np.ascontiguousarray(
        w3[:, :, :TM * 128].reshape(F, P, TM, 128).transpose(3, 1, 2, 0)
    ).astype(bf16)                                # (128, P, TM, F)

    b2 = np.asarray(bias, np.float32).reshape(F, P)
    d3 = np.asarray(dec_w, np.float32).reshape(OUT, F, P)

    in_maps = []
    for core in range(NCORES):
        bh, q4 = core // 4, core % 4
        p0 = q4 * NP
        xs = np.ascontiguousarray(xm[bh, :, p0:p0 + NP]).reshape(
            128, NP * TM * NB)
        ws = np.ascontiguousarray(wm[:, p0:p0 + NP]).reshape(
            128, NP * TM * F)
        xtl = np.zeros((128, NG * NB), bf16)
        wtl = np.zeros((128, NP * F), bf16)
        bst = np.zeros((128, NG), np.float32)
        dst = np.zeros((128, NG * OUT), np.float32)
        for pl in range(NP):
            G, q = pl // 4, pl % 4
            xtl[32 * q:32 * q + TAIL, G * NB:(G + 1) * NB] = tl[bh, p0 + pl]
            wtl[32 * q:32 * q + TAIL, pl * F:(pl + 1) * F] = \
                w3[:, p0 + pl, TM * 128:].T.astype(bf16)
            bst[32 * q:32 * q + F, G] = b2[:, p0 + pl]
            dst[32 * q:32 * q + F, G * OUT:(G + 1) * OUT] = d3[:, :, p0 + pl].T
        in_maps.append(
            {"x": xs.view(np.float32), "xt": xtl.view(np.float32),
             "w": ws.view(np.float32), "wt": wtl.view(np.float32),
             "bias": bst, "dec": dst.astype(bf16)})
    return in_maps


def combine(results, dec_b):
    dec_b = np.asarray(dec_b, np.float32)
    out = np.zeros((B, OUT), np.float32)
    for bh in range(2):
        part = results[4 * bh]["out"].astype(np.float32)
        for q4 in range(1, 4):
            part = part + results[4 * bh + q4]["out"]
        out[bh * NB:(bh + 1) * NB] = part.T + dec_b
    return out


_cache = {}


def _get_nc():
    if "nc" not in _cache:
        nc = build_program()
        nc.finalize()
        _cache["nc"] = nc
    return _cache["nc"]


def _install_ntff_hook():
    """Provide the missing antenv.axon_hooks module so trace=True works
    under axon (replicates trn_boot._ntff_profile_via_ctypes)."""
    import contextlib
    import ctypes
    import types

    if "antenv.axon_hooks" in sys.modules:
        return
    so_path = "/opt/axon/libaxon_pjrt.so"
    holder = {}
    mod = types.ModuleType("antenv.axon_hooks")
    mod.set_axon_ntff_profile_hook = lambda h: holder.__setitem__("h", h)
    mod.get_axon_ntff_profile_hook = lambda: holder.get("h")
    sys.modules["antenv.axon_hooks"] = mod
    try:
        import antenv
        antenv.axon_hooks = mod
    except ImportError:
        pass

    lib = ctypes.CDLL(so_path)
    if not hasattr(lib, "axon_start_nrt_profile"):
        return
    lib.axon_start_nrt_profile.argtypes = [
        ctypes.POINTER(ctypes.c_int64), ctypes.c_size_t]
    lib.axon_start_nrt_profile.restype = ctypes.c_int64
    lib.axon_stop_nrt_profile.argtypes = [ctypes.c_char_p]
    lib.axon_stop_nrt_profile.restype = ctypes.c_int64

    @contextlib.contextmanager
    def _hook(output_dir, device_ids):
        import jax
        jax.devices()
        if device_ids:
            ids = (ctypes.c_int64 * len(device_ids))(*device_ids)
            rc = lib.axon_start_nrt_profile(ids, len(device_ids))
        else:
            rc = lib.axon_start_nrt_profile(None, 0)
        if rc != 0:
            raise RuntimeError(f"axon_start_nrt_profile rc={rc}")
        try:
            yield
        finally:
            n = lib.axon_stop_nrt_profile(str(output_dir).encode())
            print(f"profile: {n} file(s) written to {output_dir}")

    mod.set_axon_ntff_profile_hook(_hook)


def run(x, weight, bias, dec_w, dec_b, trace=False):
    from concourse import bass_utils
    from concourse.bass_utils import run_bass_kernel_spmd

    if trace:
        _install_ntff_hook()
        # artifact upload needs a bucket that doesn't exist here
        bass_utils.upload_artifacts = lambda tmpdir: tmpdir

    nc = _get_nc()
    in_maps = make_in_maps(x, weight, bias, dec_w)
    r = run_bass_kernel_spmd(nc, in_maps, list(range(NCORES)), trace=trace)
    return combine(r.results, dec_b), r


def kernel(x, weight, bias, dec_w, dec_b):
    out, _ = run(x, weight, bias, dec_w, dec_b, trace=False)
    return out


# revision 20
# speedup vs baseline: 1.7293x; 1.0473x over previous
"""Trainium2 Bass kernel for nn_LCN (locally-connected network).

Computation:
  x: (512, 1, 280, 280) -> non-overlapping 28x28 patches (10x10 grid, P=100)
  y[b, f, p] = sum_{k,l} x[b, 28ph+k, 28pw+l] * w[f*100+p, 0, k, l]
  y = relu(y + bias[f*100+p]);  out = y_flat @ dec_w.T + dec_b   (j = f*100 + p)

Strategy (v3): the problem is HBM-stream-bound, so minimize bytes moved and
keep the device program trivial:
  - Host casts x to bf16 (error budget 2e-2 >> bf16's ~5e-3; PE computed in
    bf16 anyway) -> halves the dominant x stream vs fp32.
  - Host pre-transposes x into contraction-major chunk layout so the PE
    does NO transposes / im2col: just matmuls, straight off the DMA stream.
  - DMA details that matter (measured): bf16-typed DMA runs at half the
    AXI beat width, so all streams are f32-typed and bitcast to bf16 for
    the PE; and only 128-partition-row DMAs align the 16 SDMA engines 1:1
    with the 16 SBUF AXI ports (112 rows -> pairwise port contention,
    16 vs 25 GB/s/engine). Hence each patch's 784 pixels are split as
    6 full K=128 chunks + a 16-pixel tail; tails of 4 patches pack into
    the 4 32-row strips of one column block and are applied by diagonal
    row+col-tiled K=16 matmuls.
  - Sharding: 2 batch halves (256 images) x 4 patch quarters (25 patches).
    Per core: x 9.8+0.5 MB + w 0.7 MB -> ~32us at ~343 GB/s/NC measured.
  - Mains: per patch 6 accumulating bf16 matmuls lhsT=w[128,16],
    rhs=x[128,256] -> y PSUM, 4 patches per PSUM tile at col offsets
    0/32/64/96 (tile_position -> concurrent col-tile streams), + 1 tail
    matmul lhsT=wt[16,16] at tile_position (32q,32q).
  - ACT: relu(y + bias) -> y_sb bf16; decoder matmul per group deferred by
    one group so the relu never stalls the PE; accumulated in PSUM.
  - The 1-patch group is processed first so the last DMA slice finishes
    into a wide 4-way-concurrent matmul tail.
Host sums the 4 patch-quarter partial decoder outputs and adds dec_b.
"""

import sys

import numpy as np

for _p in ("/opt/trn_rl_repo", "/opt/trn_rl_repo/concourse"):
    if _p not in sys.path:
        sys.path.insert(0, _p)

import concourse.bass as bass
import concourse.mybir as mybir
import concourse.tile as tile
from concourse import bacc

F32 = mybir.dt.float32
BF16 = mybir.dt.bfloat16

# Problem constants
B, H, W = 512, 280, 280
KS = 28
HS = WS = 10
P = 100         # patches per image
F = 16
OUT = 10
NCORES = 8

# Sharding: 2 batch halves x 4 patch quarters
NB = 256        # images per core
NP = 25         # patches per core
TM = 6          # full K=128 chunks per patch (768 of 784 pixels)
TAIL = 16       # leftover pixels per patch (pixel 768..783)
NG = (NP + 3) // 4  # PSUM groups of 4 patches (last group: 1 patch)
YB = 3          # y PSUM bufs


def build_program():
    nc = bacc.Bacc("TRN2")
    # All streams are f32-typed (bf16 data, bitcast on the PE side) and use
    # 128 partition rows -- both required for full DMA rate (see docstring).
    # Everything that isn't the main x stream (w, w-tail, bias, dec, x-tail)
    # is packed into ONE const tensor: separate small DMAs decay into
    # 128 descriptor-dominated packets that crawl behind the x stream and
    # stall the in-order PE queue at the first decoder matmul.
    CW = NP * TM * F // 2       # 1200  w main (f32 cols)
    CWT = NP * F // 2           # 200   w tail
    CB = NG                     # 7     bias (real f32)
    CD = NG * OUT // 2          # 35    dec (bf16 pairs)
    CXT = NG * NB // 2          # 896   x tail
    CC = CW + CWT + CB + CD + CXT
    c_d = nc.dram_tensor("consts", [128, CC], F32, kind="ExternalInput")
    x_d = nc.dram_tensor("x", [128, NP * TM * NB // 2], F32, kind="ExternalInput")
    o_d = nc.dram_tensor("out", [OUT, NB], F32, kind="ExternalOutput")

    with tile.TileContext(nc) as tc:
        with (
            tc.tile_pool(name="const", bufs=1) as constp,
            tc.tile_pool(name="xs", bufs=1) as xsp,
            tc.tile_pool(name="yps", bufs=YB, space="PSUM") as ypsp,
            tc.tile_pool(name="ops", bufs=1, space="PSUM") as opsp,
        ):
            c_sb = constp.tile([128, CC], F32)
            y_sb = constp.tile([128, NG * NB], BF16)
            zero_sb = constp.tile([128, NB], F32)
            out_sb = constp.tile([OUT, NB], F32)
            out_ps = opsp.tile([OUT, NB], F32)

            # consts first on the sync ring: one efficient wide DMA
            nc.sync.dma_start(out=c_sb[:], in_=c_d[:])
            nc.gpsimd.memset(zero_sb[:], 0.0)

            # groups processed 1-patch group first, so the final DMA slice
            # drains into a wide concurrent matmul tail
            order = [NG - 1] + list(range(NG - 1))

            x_tiles = {}
            for g in order:
                npg = min(4, NP - 4 * g)
                t = xsp.tile([128, npg * TM * NB // 2], F32, name=f"xg{g}")
                off = 4 * g * TM * NB // 2
                nc.sync.dma_start(
                    out=t[:], in_=x_d[:, off:off + npg * TM * NB // 2])
                x_tiles[g] = t
            w_bf = c_sb[:, 0:CW].bitcast(BF16)
            wt_bf = c_sb[:, CW:CW + CWT].bitcast(BF16)
            bias_sb = c_sb[:, CW + CWT:CW + CWT + CB]
            dec_bf = c_sb[:, CW + CWT + CB:CW + CWT + CB + CD].bitcast(BF16)
            xt_bf = c_sb[:, CW + CWT + CB + CD:CC].bitcast(BF16)

            def emit_dec(g, first, last):
                nc.tensor.matmul(
                    out_ps[:],
                    dec_bf[:, g * OUT:(g + 1) * OUT],
                    y_sb[:, g * NB:(g + 1) * NB],
                    start=first,
                    stop=last,
                )

            prev_g = None
            for idx, g in enumerate(order):
                npg = min(4, NP - 4 * g)
                yt = ypsp.tile([128, NB], F32, name="y_ps")
                if idx < YB:
                    # clear stale/NaN PSUM so the gap rows are finite zeros
                    nc.vector.tensor_copy(yt[:], zero_sb[:])
                xg_bf = x_tiles[g][:].bitcast(BF16)
                for t in range(TM):
                    for q in range(npg):
                        pl = 4 * g + q
                        nc.tensor.matmul(
                            yt[32 * q:32 * q + F, :],
                            w_bf[:, (pl * TM + t) * F:(pl * TM + t + 1) * F],
                            xg_bf[:, (q * TM + t) * NB:(q * TM + t + 1) * NB],
                            start=(t == 0),
                            stop=False,
                            tile_position=(0, 32 * q),
                        )
                for q in range(npg):
                    # 16-pixel tail: diagonal row+col tile (32q, 32q)
                    pl = 4 * g + q
                    blk = pl // 4
                    nc.tensor.matmul(
                        yt[32 * q:32 * q + F, :],
                        wt_bf[32 * q:32 * q + TAIL, pl * F:(pl + 1) * F],
                        xt_bf[32 * q:32 * q + TAIL, blk * NB:(blk + 1) * NB],
                        start=False,
                        stop=True,
                        tile_position=(32 * q, 32 * q),
                    )
                nc.scalar.activation(
                    out=y_sb[:, g * NB:(g + 1) * NB],
                    in_=yt[:],
                    func=mybir.ActivationFunctionType.Relu,
                    bias=bias_sb[:, g:g + 1],
                )  # noqa: bias slice of c_sb
                # decoder deferred one group so its y_sb dependency (the
                # relu above) never stalls the PE behind independent mains
                if prev_g is not None:
                    emit_dec(prev_g, prev_g == order[0], False)
                prev_g = g
            emit_dec(prev_g, False, True)

            nc.vector.tensor_copy(out_sb[:], out_ps[:])
            nc.sync.dma_start(out=o_d[:], in_=out_sb[:])

    return nc


def make_in_maps(x, weight, bias, dec_w):
    import ml_dtypes
    bf16 = ml_dtypes.bfloat16

    x = np.asarray(x, np.float32).reshape(B, H, W)
    xb = x.astype(bf16)
    # (bh, b, hs, k, ws, l) -> (bh, hs, ws, k, l, b): per-patch pixel-major
    x6 = xb.reshape(2, NB, HS, KS, WS, KS)
    pp = np.ascontiguousarray(x6.transpose(0, 2, 4, 3, 5, 1)).reshape(
        2, P, KS * KS, NB)
    # main chunks: pixel j in [0,768) -> row j%128, col (p, t=j//128, b)
    xm = np.ascontiguousarray(
        pp[:, :, :TM * 128].reshape(2, P, TM, 128, NB).transpose(0, 3, 1, 2, 4))
    tl = pp[:, :, TM * 128:]                      # (2, P, 16, NB)

    # weight row j = f*100 + p (pixel-major k*28+l)
    w3 = np.asarray(weight, np.float32).reshape(F, P, KS * KS)
    wm = np.ascontiguousarray(
        w3[:, :, :TM * 128].reshape(F, P, TM, 128).transpose(3, 1, 2, 0)
    ).astype(bf16)                                # (128, P, TM, F)

    b2 = np.asarray(bias, np.float32).reshape(F, P)
    d3 = np.asarray(dec_w, np.float32).reshape(OUT, F, P)

    in_maps = []
    for core in range(NCORES):
        bh, q4 = core // 4, core % 4
        p0 = q4 * NP
        xs = np.ascontiguousarray(xm[bh, :, p0:p0 + NP]).reshape(
            128, NP * TM * NB)
        ws = np.ascontiguousarray(wm[:, p0:p0 + NP]).reshape(
            128, NP * TM * F)
        xtl = np.zeros((128, NG * NB), bf16)
        wtl = np.zeros((128, NP * F), bf16)
        bst = np.zeros((128, NG), np.float32)
        dst = np.zeros((128, NG * OUT), np.float32)
        for pl in range(NP):
            G, q = pl // 4, pl % 4
            xtl[32 * q:32 * q + TAIL, G * NB:(G + 1) * NB] = tl[bh, p0 + pl]
            wtl[32 * q:32 * q + TAIL, pl * F:(pl + 1) * F] = \
                w3[:, p0 + pl, TM * 128:].T.astype(bf16)
            bst[32 * q:32 * q + F, G] = b2[:, p0 + pl]
            dst[32 * q:32 * q + F, G * OUT:(G + 1) * OUT] = d3[:, :, p0 + pl].T
        consts = np.concatenate(
            [ws.view(np.float32), wtl.view(np.float32), bst,
             dst.astype(bf16).view(np.float32), xtl.view(np.float32)],
            axis=1)
        in_maps.append({"consts": consts, "x": xs.view(np.float32)})
    return in_maps


def combine(results, dec_b):
    dec_b = np.asarray(dec_b, np.float32)
    out = np.zeros((B, OUT), np.float32)
    for bh in range(2):
        part = results[4 * bh]["out"].astype(np.float32)
        for q4 in range(1, 4):
            part = part + results[4 * bh + q4]["out"]
        out[bh * NB:(bh + 1) * NB] = part.T + dec_b
    return out


_cache = {}


def _get_nc():
    if "nc" not in _cache:
        nc = build_program()
        nc.finalize()
        _cache["nc"] = nc
    return _cache["nc"]


def _install_ntff_hook():
    """Provide the missing antenv.axon_hooks module so trace=True works
    under axon (replicates trn_boot._ntff_profile_via_ctypes)."""
    import contextlib
    import ctypes
    import types

    if "antenv.axon_hooks" in sys.modules:
        return
    so_path = "/opt/axon/libaxon_pjrt.so"
    holder = {}
    mod = types.ModuleType("antenv.axon_hooks")
    mod.set_axon_ntff_profile_hook = lambda h: holder.__setitem__("h", h)
    mod.get_axon_ntff_profile_hook = lambda: holder.get("h")
    sys.modules["antenv.axon_hooks"] = mod
    try:
        import antenv
        antenv.axon_hooks = mod
    except ImportError:
        pass

    lib = ctypes.CDLL(so_path)
    if not hasattr(lib, "axon_start_nrt_profile"):
        return
    lib.axon_start_nrt_profile.argtypes = [
        ctypes.POINTER(ctypes.c_int64), ctypes.c_size_t]
    lib.axon_start_nrt_profile.restype = ctypes.c_int64
    lib.axon_stop_nrt_profile.argtypes = [ctypes.c_char_p]
    lib.axon_stop_nrt_profile.restype = ctypes.c_int64

    @contextlib.contextmanager
    def _hook(output_dir, device_ids):
        import jax
        jax.devices()
        if device_ids:
            ids = (ctypes.c_int64 * len(device_ids))(*device_ids)
            rc = lib.axon_start_nrt_profile(ids, len(device_ids))
        else:
            rc = lib.axon_start_nrt_profile(None, 0)
        if rc != 0:
            raise RuntimeError(f"axon_start_nrt_profile rc={rc}")
        try:
            yield
        finally:
            n = lib.axon_stop_nrt_profile(str(output_dir).encode())
            print(f"profile: {n} file(s) written to {output_dir}")

    mod.set_axon_ntff_profile_hook(_hook)


def run(x, weight, bias, dec_w, dec_b, trace=False):
    from concourse import bass_utils
    from concourse.bass_utils import run_bass_kernel_spmd

    if trace:
        _install_ntff_hook()
        # artifact upload needs a bucket that doesn't exist here
        bass_utils.upload_artifacts = lambda tmpdir: tmpdir

    nc = _get_nc()
    in_maps = make_in_maps(x, weight, bias, dec_w)
    r = run_bass_kernel_spmd(nc, in_maps, list(range(NCORES)), trace=trace)
    return combine(r.results, dec_b), r


def kernel(x, weight, bias, dec_w, dec_b):
    out, _ = run(x, weight, bias, dec_w, dec_b, trace=False)
    return out
